# revision 10
# baseline (speedup 1.0000x reference)
"""GAT (3-layer, PyG-style) on 8 Trainium2 NeuronCores via Bass/Tile.

Strategy (dst-sharded graph parallel):
  - Nodes sharded 8 ways by destination; edges partitioned by dst shard,
    sorted by dst, grouped into 128-dst "tiles" and 128-edge "chunks".
  - Per layer, a node table [N, row] holds [h(bf16) | a_src(f32) | a_dst(f32)]
    in HBM on every core (stitched on host between launches = halo exchange).
  - Per-edge source rows fetched with gpsimd dma_gather (int16 idx, low/high
    table-half split to fit int16).
  - a_dst[dst] expanded per-edge with a host-built one-hot S_T (fp8) matmul.
  - logits -> Prelu -> Exp on ACT; e * h[src] on DVE; segment-sum via
    host-built one-hot S (fp8) matmuls into PSUM, with e appended as extra
    rhs columns so segment softmax denominators come out of the same matmul.
  - Per-node epilogue: normalize, +bias, ELU, transpose, next-layer matmul
    (W pre-augmented with attention vectors so al_src/al_dst ride along).
"""
import numpy as np
import ml_dtypes
from contextlib import ExitStack

import concourse.bass as bass
import concourse.tile as tile
from concourse import bacc, mybir
from concourse import bass_utils
from concourse.masks import make_identity

P = 128
N_NODES = 50000
N_EDGES = 650000
NEG_SLOPE = 0.2
N_CORES = 8
NS = N_NODES // N_CORES            # 6250 nodes per shard
NT = (NS + P - 1) // P             # 49 dst tiles per core
NSP = NT * P                       # padded shard nodes (6272)
HALF = 32768                       # int16 gather limit -> low/high split
ROWW = 384                         # bf16 slots per table row (768B)
ROWW3 = 64                         # f32 per layer-3 table row (256B)
F = 256                            # feature width (H*C)
H = 4

FP8 = mybir.dt.float8e4
BF16 = mybir.dt.bfloat16
FP16 = mybir.dt.float16
F32 = mybir.dt.float32
I16 = mybir.dt.int16


# ----------------------------------------------------------------- host prep

def _wrap16(idx_flat):
    """[n] int array -> [128, n//16] int16 (16-partition wrap, replicated)."""
    n = len(idx_flat)
    a = np.asarray(idx_flat, dtype=np.int16).reshape(n // 16, 16).T
    return np.tile(a, (8, 1))


def build_schedule(src, dst):
    """Partition edges by dst shard / dst tile / src half, pad to chunks."""
    order = np.argsort(dst, kind="stable")
    src = src[order]
    dst = dst[order]
    core_of = dst // NS
    core_starts = np.searchsorted(core_of, np.arange(N_CORES + 1))

    per_core = []
    for c in range(N_CORES):
        s0, s1 = core_starts[c], core_starts[c + 1]
        cs, cd = src[s0:s1], dst[s0:s1] - c * NS
        tile_of = cd // P
        tile_starts = np.searchsorted(tile_of, np.arange(NT + 1))
        tiles = []
        for t in range(NT):
            t0, t1 = tile_starts[t], tile_starts[t + 1]
            ts_, td_ = cs[t0:t1], cd[t0:t1] - t * P
            lo = ts_ < HALF
            tiles.append(((ts_[lo], td_[lo]), (ts_[~lo] - HALF, td_[~lo])))
        per_core.append(tiles)

    LCH = np.zeros(NT, np.int64)
    HCH = np.zeros(NT, np.int64)
    for t in range(NT):
        for c in range(N_CORES):
            (ls, _), (hs, _) = per_core[c][t]
            LCH[t] = max(LCH[t], -(-len(ls) // P))
            HCH[t] = max(HCH[t], -(-len(hs) // P))
        LCH[t] = max(LCH[t], 1)
    NCH = LCH + HCH
    ch0 = np.concatenate([[0], np.cumsum(NCH)])
    TOTCH = int(ch0[-1])

    idx16 = np.zeros((N_CORES, P, TOTCH * 8), np.int16)
    S = np.zeros((N_CORES, P, TOTCH, P), ml_dtypes.float8_e4m3)
    ST = np.zeros((N_CORES, P, TOTCH, P), ml_dtypes.float8_e4m3)
    one = ml_dtypes.float8_e4m3(1.0)
    for c in range(N_CORES):
        for t in range(NT):
            for half, (es, ed) in enumerate(per_core[c][t]):
                nch = int((LCH[t], HCH[t])[half])
                if nch == 0:
                    continue
                base = int(ch0[t]) + (int(LCH[t]) if half else 0)
                n = nch * P
                e_pad = np.zeros(n, np.int64)
                e_pad[: len(es)] = es
                if len(es):
                    k = np.arange(len(es))
                    S[c, k % P, base + k // P, ed] = one
                    ST[c, ed, base + k // P, k % P] = one
                idx16[c, :, base * 8 : (base + nch) * 8] = _wrap16(e_pad)
    return dict(LCH=LCH, HCH=HCH, NCH=NCH, ch0=ch0, TOTCH=TOTCH,
                idx16=idx16, S=S, ST=ST)


# ------------------------------------------------------------- bass builders

class Prog:
    def __init__(self):
        self.nc = bacc.Bacc("TRN2", target_bir_lowering=False, debug=False,
                            num_devices=N_CORES)
        self.in_aps = {}
        self.out_aps = {}

    def inp(self, name, shape, dt):
        ap = self.nc.dram_tensor(name, list(shape), dt, kind="ExternalInput").ap()
        self.in_aps[name] = ap
        return ap

    def out(self, name, shape, dt):
        ap = self.nc.dram_tensor(name, list(shape), dt, kind="ExternalOutput").ap()
        self.out_aps[name] = ap
        return ap


def _table_write(nc, sb, h_ps, tabout, t):
    """PSUM [128, 264] -> table row tile (bf16 h + raw f32 as/ad) -> HBM."""
    trow = sb.tile([P, ROWW], BF16, tag="trow")
    nc.vector.memset(trow[:, F + 16 : ROWW], 0.0)
    nc.scalar.copy(trow[:, 0:F], h_ps[:, 0:F])
    nc.vector.tensor_copy(trow[:, F : F + 16].bitcast(F32), h_ps[:, F : F + 8])
    nc.sync.dma_start(tabout[t * P : (t + 1) * P, :], trow[:])


def build_launch_A():
    """x_shard @ Waug1 -> table1 rows [h1 | as1 | ad1]."""
    pr = Prog()
    nc = pr.nc
    x = pr.inp("x", [NSP, P], F32)
    w1 = pr.inp("w1", [P, F + 8], F32)
    tab = pr.out("tab", [NSP, ROWW], BF16)
    with tile.TileContext(nc) as tc, ExitStack() as ctx:
        sb = ctx.enter_context(tc.tile_pool(name="sb", bufs=3))
        ps = ctx.enter_context(tc.tile_pool(name="ps", bufs=3, space="PSUM"))
        cpool = ctx.enter_context(tc.tile_pool(name="cp", bufs=1))
        ident = cpool.tile([P, P], F32)
        make_identity(nc, ident[:])
        w1t = cpool.tile([P, F + 8], F32)
        nc.sync.dma_start(w1t[:], w1)
        for t in range(NT):
            xt = sb.tile([P, P], F32, tag="xt")
            nc.sync.dma_start(xt[:], x[t * P : (t + 1) * P, :])
            xT_ps = ps.tile([P, P], F32, space="PSUM", tag="xT")
            nc.tensor.transpose(xT_ps[:], xt[:], ident[:])
            xT = sb.tile([P, P], F32, tag="xTs")
            nc.scalar.copy(xT[:], xT_ps[:])
            h_ps = ps.tile([P, F + 8], F32, space="PSUM", tag="hps")
            nc.tensor.matmul(h_ps[:], lhsT=xT[:], rhs=w1t[:], start=True, stop=True)
            _table_write(nc, sb, h_ps, tab, t)
    nc.compile()
    return pr


def build_launch_agg(sch, layer, b3=0.0, stage=99):
    """layer=1: L1 agg -> table2 (bf16 rows); layer=2: L2 agg -> table3
    (f32 rows); layer=3: L3 agg -> output.  stage: debug early-exit level."""
    pr = Prog()
    nc = pr.nc
    TOTCH = sch["TOTCH"]
    last = layer == 3
    roww = ROWW3 if last else ROWW
    tab_dt = F32 if last else BF16
    table = pr.inp("table", [N_NODES, roww], tab_dt)
    mytab = pr.inp("mytab", [NSP, roww], tab_dt)
    idx16 = pr.inp("idx16", [P, TOTCH * 8], I16)
    S_in = pr.inp("S", [P, TOTCH, P], FP8)
    ST_in = pr.inp("ST", [P, TOTCH, P], FP8)
    if layer == 1:
        naug = F + 8
        waug = pr.inp("waug", [F, naug], F32)
        tabout = pr.out("tabout", [NSP, ROWW], BF16)
    elif layer == 2:
        naug = 3
        waug = pr.inp("waug", [F, naug], F32)
        tabout = pr.out("tabout", [NSP, ROWW3], F32)
    else:
        outv = pr.out("outv", [NSP, 1], F32)
    if not last:
        wcol = pr.inp("wcol", [1, naug], F32)
        bias = pr.inp("bias", [P, 2], F32)
        nbias = pr.inp("nbias", [P, 2], F32)

    NAGG = (F + 4) if not last else 2
    nad = H if not last else 1
    with tile.TileContext(nc) as tc, ExitStack() as ctx:
        sb = ctx.enter_context(tc.tile_pool(name="sb", bufs=2))
        sbg = ctx.enter_context(tc.tile_pool(name="sbg", bufs=2))
        ps = ctx.enter_context(tc.tile_pool(name="ps", bufs=2, space="PSUM"))
        psa = ctx.enter_context(tc.tile_pool(name="psa", bufs=2, space="PSUM"))
        cpool = ctx.enter_context(tc.tile_pool(name="cp", bufs=1))
        ident = cpool.tile([P, P], F32)
        make_identity(nc, ident[:])
        if not last:
            waug_t = cpool.tile([P, F // P, naug], F32, tag="waug")
            for k in range(F // P):
                nc.sync.dma_start(waug_t[:, k, :], waug[k * P : (k + 1) * P, :])
            wcol_t = cpool.tile([1, naug], F32, tag="wcol")
            nc.sync.dma_start(wcol_t[:], wcol)
            bias_t = cpool.tile([P, 2], F32, tag="bias")
            nc.sync.dma_start(bias_t[:], bias)
            nbias_t = cpool.tile([P, 2], F32, tag="nbias")
            nc.sync.dma_start(nbias_t[:], nbias)
            negone = cpool.tile([1, P], F32, tag="negone")
            nc.vector.memset(negone[:], -1.0)

        for t in range(NT):
            NCH = int(sch["NCH"][t])
            LCH = int(sch["LCH"][t])
            HCH = int(sch["HCH"][t])
            c0 = int(sch["ch0"][t])
            idx_t = sb.tile([P, NCH * 8], I16, tag="idx")
            nc.sync.dma_start(idx_t[:], idx16[:, c0 * 8 : (c0 + NCH) * 8])
            s_t = sb.tile([P, NCH, P], FP8, tag="S")
            nc.sync.dma_start(s_t[:], S_in[:, c0 : c0 + NCH, :])
            st_t = sb.tile([P, NCH, P], FP8, tag="ST")
            nc.sync.dma_start(st_t[:], ST_in[:, c0 : c0 + NCH, :])
            g_t = sbg.tile([P, NCH, roww], tab_dt, tag="G")
            GMAX = 8  # 1024-descriptor SWDGE ring cap per dma_gather
            for a0, a1, base in ((0, LCH, 0), (LCH, NCH, HALF)):
                for j0 in range(a0, a1, GMAX):
                    j1 = min(j0 + GMAX, a1)
                    nc.gpsimd.dma_gather(
                        out_ap=g_t[:, j0:j1, :],
                        in_ap=table if base == 0 else table[base:, :],
                        idxs_ap=idx_t[:, j0 * 8 : j1 * 8],
                        num_idxs=(j1 - j0) * P, num_idxs_reg=(j1 - j0) * P,
                        elem_size=roww)
            # adtile: own-shard a_dst rows for this tile, cast to fp16
            if not last:
                adraw = sb.tile([P, 16], BF16, tag="adraw")
                nc.sync.dma_start(adraw[:], mytab[t * P : (t + 1) * P, F : F + 16])
                ad_f32 = adraw[:].bitcast(F32)[:, 4:8]
            else:
                adraw = sb.tile([P, 4], F32, tag="adraw")
                nc.sync.dma_start(adraw[:], mytab[t * P : (t + 1) * P, 0:4])
                ad_f32 = adraw[:, 2:3]
            adt = sb.tile([P, nad], FP16, tag="adt")
            nc.vector.tensor_copy(adt[:], ad_f32)
            if stage == 0:
                trow = sb.tile([P, ROWW if not last else ROWW3], tab_dt, tag="trow")
                nc.vector.tensor_copy(trow[:], g_t[:, 0, :])
                nc.sync.dma_start(tabout[t * P : (t + 1) * P, :], trow[:])
                continue
            # a_dst expansion matmuls (per chunk) into one PSUM strip
            zps = ps.tile([P, NCH * nad], F32, space="PSUM", tag="zps")
            for j in range(NCH):
                nc.tensor.matmul(zps[:, j * nad : (j + 1) * nad],
                                 lhsT=st_t[:, j, :], rhs=adt[:],
                                 start=True, stop=True)
            # z = a_src + expanded a_dst ; e = exp(prelu(z))
            if not last:
                as_ap = g_t[:, :, F : F + 16].bitcast(F32)[:, :, 0:4]
            else:
                as_ap = g_t[:, :, 1:2]
            z_t = sb.tile([P, NCH, nad], F32, tag="z")
            nc.vector.tensor_tensor(
                out=z_t[:], in0=as_ap,
                in1=zps[:].rearrange("p (c h) -> p c h", h=nad),
                op=mybir.AluOpType.add)
            l_t = sb.tile([P, NCH, nad], F32, tag="l")
            nc.scalar.activation(l_t[:], z_t[:],
                                 mybir.ActivationFunctionType.Prelu,
                                 alpha=NEG_SLOPE)
            e_t = sb.tile([P, NCH, nad], F32, tag="e")
            nc.scalar.activation(e_t[:], l_t[:],
                                 mybir.ActivationFunctionType.Exp)
            if stage == 1:
                trow = sb.tile([P, ROWW if not last else ROWW3], tab_dt, tag="trow")
                nc.vector.memset(trow[:], 0.0)
                nc.vector.tensor_copy(trow[:, 0 : NCH * nad], e_t[:])
                nc.sync.dma_start(tabout[t * P : (t + 1) * P, :], trow[:])
                continue
            # weighted messages rhs = [e*h | e]
            eg_t = sbg.tile([P, NCH, NAGG], BF16, tag="eg")
            if not last:
                nc.vector.tensor_tensor(
                    out=eg_t[:, :, 0:F].rearrange("p c (h f) -> p c h f", h=H),
                    in0=g_t[:, :, 0:F].rearrange("p c (h f) -> p c h f", h=H),
                    in1=e_t[:].broadcast_to([P, NCH, H, F // H]),
                    op=mybir.AluOpType.mult)
                nc.vector.tensor_copy(eg_t[:, :, F : F + 4], e_t[:])
            else:
                nc.vector.tensor_tensor(
                    out=eg_t[:, :, 0:1], in0=g_t[:, :, 0:1], in1=e_t[:],
                    op=mybir.AluOpType.mult)
                nc.vector.tensor_copy(eg_t[:, :, 1:2], e_t[:])
            if stage == 2:
                trow = sb.tile([P, ROWW if not last else ROWW3], tab_dt, tag="trow")
                nc.vector.tensor_copy(trow[:, 0:NAGG], eg_t[:, 0, :])
                nc.vector.memset(trow[:, NAGG:], 0.0)
                nc.sync.dma_start(tabout[t * P : (t + 1) * P, :], trow[:])
                continue
            # aggregation matmuls
            agg = psa.tile([P, NAGG], F32, space="PSUM", tag="agg")
            for j in range(NCH):
                nc.tensor.matmul(agg[:], lhsT=s_t[:, j, :], rhs=eg_t[:, j, :],
                                 start=(j == 0), stop=(j == NCH - 1))
            if stage == 3:
                trow = sb.tile([P, ROWW if not last else ROWW3], tab_dt, tag="trow")
                nc.vector.tensor_copy(trow[:, 0:NAGG], agg[:])
                nc.vector.memset(trow[:, NAGG:], 0.0)
                nc.sync.dma_start(tabout[t * P : (t + 1) * P, :], trow[:])
                continue
            # epilogue
            if last:
                den = sb.tile([P, 1], F32, tag="den")
                nc.vector.tensor_scalar_add(den[:], agg[:, 1:2], 1e-16)
                r_t = sb.tile([P, 1], F32, tag="r")
                nc.vector.reciprocal(r_t[:], den[:])
                o_t = sb.tile([P, 1], F32, tag="o")
                nc.vector.tensor_tensor(out=o_t[:], in0=agg[:, 0:1], in1=r_t[:],
                                        op=mybir.AluOpType.mult)
                if b3 != 0.0:
                    nc.vector.tensor_scalar_add(o_t[:], o_t[:], float(b3))
                nc.sync.dma_start(outv[t * P : (t + 1) * P, :], o_t[:])
                continue
            den = sb.tile([P, H], F32, tag="den")
            nc.vector.tensor_scalar_add(den[:], agg[:, F : F + 4], 1e-16)
            r_t = sb.tile([P, H], F32, tag="r")
            nc.vector.reciprocal(r_t[:], den[:])
            xn = sb.tile([P, F], F32, tag="xn")
            for h in range(H):
                nc.scalar.mul(xn[:, h * 64 : (h + 1) * 64],
                              agg[:, h * 64 : (h + 1) * 64], r_t[:, h : h + 1])
            # ELU(x + b) = relu(z+b) + exp(min(z+b,0)) - 1, -1 folded into
            # matmul via negone row; done on transposed tiles (bias per part.)
            h_ps = psa.tile([P, naug], F32, space="PSUM", tag="hps")
            for k in range(2):
                xT_ps = ps.tile([P, P], F32, space="PSUM", tag="xT")
                nc.tensor.transpose(xT_ps[:], xn[:, k * P : (k + 1) * P], ident[:])
                p_t = sb.tile([P, P], F32, tag="p")
                nc.scalar.activation(p_t[:], xT_ps[:],
                                     mybir.ActivationFunctionType.Relu,
                                     bias=bias_t[:, k : k + 1])
                m_t = sb.tile([P, P], F32, tag="m")
                nc.scalar.activation(m_t[:], xT_ps[:],
                                     mybir.ActivationFunctionType.Relu,
                                     bias=nbias_t[:, k : k + 1], scale=-1.0)
                q_t = sb.tile([P, P], F32, tag="q")
                nc.scalar.activation(q_t[:], m_t[:],
                                     mybir.ActivationFunctionType.Exp,
                                     scale=-1.0)
                xe_t = sb.tile([P, P], F32, tag="xe")
                nc.vector.tensor_tensor(out=xe_t[:], in0=p_t[:], in1=q_t[:],
                                        op=mybir.AluOpType.add)
                nc.tensor.matmul(h_ps[:], lhsT=xe_t[:], rhs=waug_t[:, k, :],
                                 start=(k == 0), stop=False)
            nc.tensor.matmul(h_ps[:], lhsT=negone[:], rhs=wcol_t[:],
                             start=False, stop=True)
            if layer == 1:
                _table_write(nc, sb, h_ps, tabout, t)
            else:
                trow = sb.tile([P, ROWW3], F32, tag="trow")
                nc.vector.memset(trow[:], 0.0)
                nc.vector.tensor_copy(trow[:, 0:3], h_ps[:, 0:3])
                nc.sync.dma_start(tabout[t * P : (t + 1) * P, :], trow[:])
    nc.compile()
    return pr


# --------------------------------------------------------------- the kernel

LAST_TIMES = {}


def _run(pr, in_maps, tag=None):
    if tag is not None:
        try:
            from concourse.timeline_sim import TimelineSim
            LAST_TIMES[tag] = TimelineSim(pr.nc, trace=False).simulate() / 1e9
        except Exception:
            pass
    res = bass_utils.run_bass_kernel_spmd(
        pr.nc, in_maps, core_ids=list(range(N_CORES)))
    return res.results


def _blockdiag_A(a_src, a_dst):
    Hh, C = a_src.shape
    A = np.zeros((Hh * C, 2 * Hh), np.float32)
    for h in range(Hh):
        A[h * C : (h + 1) * C, h] = a_src[h]
        A[h * C : (h + 1) * C, Hh + h] = a_dst[h]
    return A


def _pad_rows(a, n):
    out = np.zeros((n,) + a.shape[1:], a.dtype)
    out[: len(a)] = a
    return out


def kernel(x, edge_index, W1, a_src1, a_dst1, b1, W2, a_src2, a_dst2, b2,
           W3, a_src3, a_dst3, b3):
    x = np.asarray(x, np.float32)
    ei = np.asarray(edge_index)
    loops = np.arange(N_NODES, dtype=np.int64)
    src = np.concatenate([ei[0], loops]).astype(np.int64)
    dst = np.concatenate([ei[1], loops]).astype(np.int64)

    sch = build_schedule(src, dst)

    W1 = np.asarray(W1, np.float32); W2 = np.asarray(W2, np.float32)
    W3 = np.asarray(W3, np.float32)
    Waug1 = np.concatenate(
        [W1, W1 @ _blockdiag_A(np.asarray(a_src1), np.asarray(a_dst1))], 1)
    Waug2 = np.concatenate(
        [W2, W2 @ _blockdiag_A(np.asarray(a_src2), np.asarray(a_dst2))], 1)
    Waug3 = np.concatenate(
        [W3, W3 * float(np.asarray(a_src3)[0, 0]),
         W3 * float(np.asarray(a_dst3)[0, 0])], 1).astype(np.float32)
    wcol2 = Waug2.sum(0, keepdims=True).astype(np.float32)
    wcol3 = Waug3.sum(0, keepdims=True).astype(np.float32)
    b1T = np.asarray(b1, np.float32).reshape(2, P).T.copy()
    b2T = np.asarray(b2, np.float32).reshape(2, P).T.copy()

    # launch A: table1 from x
    prA = build_launch_A()
    inA = []
    for c in range(N_CORES):
        inA.append(dict(x=_pad_rows(x[c * NS : (c + 1) * NS], NSP), w1=Waug1))
    resA = _run(prA, inA, tag="A")
    tab1 = np.concatenate([resA[c]["tab"][:NS] for c in range(N_CORES)], 0)
    tab1 = np.ascontiguousarray(tab1)

    # launch B: L1 aggregation -> table2
    prB = build_launch_agg(sch, 1)
    inB = [dict(table=tab1, mytab=_pad_rows(tab1[c * NS : (c + 1) * NS], NSP),
                idx16=sch["idx16"][c], S=sch["S"][c], ST=sch["ST"][c],
                waug=Waug2, wcol=wcol2, bias=b1T, nbias=np.ascontiguousarray(-b1T))
           for c in range(N_CORES)]
    resB = _run(prB, inB, tag="B")
    tab2 = np.ascontiguousarray(
        np.concatenate([resB[c]["tabout"][:NS] for c in range(N_CORES)], 0))

    # launch C: L2 aggregation -> table3
    prC = build_launch_agg(sch, 2)
    inC = [dict(table=tab2, mytab=_pad_rows(tab2[c * NS : (c + 1) * NS], NSP),
                idx16=sch["idx16"][c], S=sch["S"][c], ST=sch["ST"][c],
                waug=Waug3, wcol=wcol3, bias=b2T, nbias=np.ascontiguousarray(-b2T))
           for c in range(N_CORES)]
    resC = _run(prC, inC, tag="C")
    tab3 = np.ascontiguousarray(
        np.concatenate([resC[c]["tabout"][:NS] for c in range(N_CORES)], 0))

    # launch D: L3 aggregation -> out
    prD = build_launch_agg(sch, 3, b3=float(np.asarray(b3).reshape(-1)[0]))
    inD = [dict(table=tab3, mytab=_pad_rows(tab3[c * NS : (c + 1) * NS], NSP),
                idx16=sch["idx16"][c], S=sch["S"][c], ST=sch["ST"][c])
           for c in range(N_CORES)]
    resD = _run(prD, inD, tag="D")
    out = np.concatenate([resD[c]["outv"][:NS] for c in range(N_CORES)], 0)
    return np.ascontiguousarray(out.astype(np.float32))


# revision 11
# speedup vs baseline: 1.0395x; 1.0395x over previous
"""GAT (3-layer, PyG-style) on 8 Trainium2 NeuronCores via Bass/Tile.

Strategy (dst-sharded graph parallel):
  - Nodes sharded 8 ways by destination; edges partitioned by dst shard,
    sorted by dst, grouped into 128-dst "tiles" and 128-edge "chunks".
  - Per layer, a node table [N, row] holds [h(bf16) | a_src(f32) | a_dst(f32)]
    in HBM on every core (stitched on host between launches = halo exchange).
  - Per-edge source rows fetched with gpsimd dma_gather (int16 idx, low/high
    table-half split to fit int16).
  - a_dst[dst] expanded per-edge with a host-built one-hot S_T (fp8) matmul.
  - logits -> Prelu -> Exp on ACT; e * h[src] on DVE; segment-sum via
    host-built one-hot S (fp8) matmuls into PSUM, with e appended as extra
    rhs columns so segment softmax denominators come out of the same matmul.
  - Per-node epilogue: normalize, +bias, ELU, transpose, next-layer matmul
    (W pre-augmented with attention vectors so al_src/al_dst ride along).
"""
import numpy as np
import ml_dtypes
from contextlib import ExitStack

import concourse.bass as bass
import concourse.tile as tile
from concourse import bacc, mybir
from concourse import bass_utils
from concourse.masks import make_identity

P = 128
N_NODES = 50000
N_EDGES = 650000
NEG_SLOPE = 0.2
N_CORES = 8
NS = N_NODES // N_CORES            # 6250 nodes per shard
NT = (NS + P - 1) // P             # 49 dst tiles per core
NSP = NT * P                       # padded shard nodes (6272)
HALF = 32768                       # int16 gather limit -> low/high split
ROWW = 384                         # bf16 slots per table row (768B)
ROWW3 = 64                         # f32 per layer-3 table row (256B)
F = 256                            # feature width (H*C)
H = 4

FP8 = mybir.dt.float8e4
BF16 = mybir.dt.bfloat16
FP16 = mybir.dt.float16
F32 = mybir.dt.float32
I16 = mybir.dt.int16


# ----------------------------------------------------------------- host prep

def _wrap16(idx_flat):
    """[n] int array -> [128, n//16] int16 (16-partition wrap, replicated)."""
    n = len(idx_flat)
    a = np.asarray(idx_flat, dtype=np.int16).reshape(n // 16, 16).T
    return np.tile(a, (8, 1))


def build_schedule(src, dst):
    """Partition edges by dst shard / dst tile / src half, pad to chunks."""
    order = np.argsort(dst, kind="stable")
    src = src[order]
    dst = dst[order]
    core_of = dst // NS
    core_starts = np.searchsorted(core_of, np.arange(N_CORES + 1))

    per_core = []
    for c in range(N_CORES):
        s0, s1 = core_starts[c], core_starts[c + 1]
        cs, cd = src[s0:s1], dst[s0:s1] - c * NS
        tile_of = cd // P
        tile_starts = np.searchsorted(tile_of, np.arange(NT + 1))
        tiles = []
        for t in range(NT):
            t0, t1 = tile_starts[t], tile_starts[t + 1]
            ts_, td_ = cs[t0:t1], cd[t0:t1] - t * P
            lo = ts_ < HALF
            tiles.append(((ts_[lo], td_[lo]), (ts_[~lo] - HALF, td_[~lo])))
        per_core.append(tiles)

    LCH = np.zeros(NT, np.int64)
    HCH = np.zeros(NT, np.int64)
    for t in range(NT):
        for c in range(N_CORES):
            (ls, _), (hs, _) = per_core[c][t]
            LCH[t] = max(LCH[t], -(-len(ls) // P))
            HCH[t] = max(HCH[t], -(-len(hs) // P))
        LCH[t] = max(LCH[t], 1)
    NCH = LCH + HCH
    ch0 = np.concatenate([[0], np.cumsum(NCH)])
    TOTCH = int(ch0[-1])

    idx16 = np.zeros((N_CORES, P, TOTCH * 8), np.int16)
    S = np.zeros((N_CORES, P, TOTCH, P), ml_dtypes.float8_e4m3)
    ST = np.zeros((N_CORES, P, TOTCH, P), ml_dtypes.float8_e4m3)
    one = ml_dtypes.float8_e4m3(1.0)
    for c in range(N_CORES):
        for t in range(NT):
            for half, (es, ed) in enumerate(per_core[c][t]):
                nch = int((LCH[t], HCH[t])[half])
                if nch == 0:
                    continue
                base = int(ch0[t]) + (int(LCH[t]) if half else 0)
                n = nch * P
                e_pad = np.zeros(n, np.int64)
                e_pad[: len(es)] = es
                if len(es):
                    k = np.arange(len(es))
                    S[c, k % P, base + k // P, ed] = one
                    ST[c, ed, base + k // P, k % P] = one
                idx16[c, :, base * 8 : (base + nch) * 8] = _wrap16(e_pad)
    return dict(LCH=LCH, HCH=HCH, NCH=NCH, ch0=ch0, TOTCH=TOTCH,
                idx16=idx16, S=S, ST=ST)


# ------------------------------------------------------------- bass builders

class Prog:
    def __init__(self):
        self.nc = bacc.Bacc("TRN2", target_bir_lowering=False, debug=False,
                            num_devices=N_CORES)
        self.in_aps = {}
        self.out_aps = {}

    def inp(self, name, shape, dt):
        ap = self.nc.dram_tensor(name, list(shape), dt, kind="ExternalInput").ap()
        self.in_aps[name] = ap
        return ap

    def out(self, name, shape, dt):
        ap = self.nc.dram_tensor(name, list(shape), dt, kind="ExternalOutput").ap()
        self.out_aps[name] = ap
        return ap


def _table_write(nc, sb, h_ps, tabout, t):
    """PSUM [128, 264] -> table row tile (bf16 h + raw f32 as/ad) -> HBM."""
    trow = sb.tile([P, ROWW], BF16, tag="trow")
    nc.vector.memset(trow[:, F + 16 : ROWW], 0.0)
    nc.scalar.copy(trow[:, 0:F], h_ps[:, 0:F])
    nc.vector.tensor_copy(trow[:, F : F + 16].bitcast(F32), h_ps[:, F : F + 8])
    nc.sync.dma_start(tabout[t * P : (t + 1) * P, :], trow[:])


def build_launch_A():
    """x_shard @ Waug1 -> table1 rows [h1 | as1 | ad1]."""
    pr = Prog()
    nc = pr.nc
    x = pr.inp("x", [NSP, P], F32)
    w1 = pr.inp("w1", [P, F + 8], F32)
    tab = pr.out("tab", [NSP, ROWW], BF16)
    with tile.TileContext(nc) as tc, ExitStack() as ctx:
        sb = ctx.enter_context(tc.tile_pool(name="sb", bufs=5))
        ps = ctx.enter_context(tc.tile_pool(name="ps", bufs=4, space="PSUM"))
        cpool = ctx.enter_context(tc.tile_pool(name="cp", bufs=1))
        ident = cpool.tile([P, P], F32)
        make_identity(nc, ident[:])
        w1t = cpool.tile([P, F + 8], F32)
        nc.sync.dma_start(w1t[:], w1)
        for t in range(NT):
            xt = sb.tile([P, P], F32, tag="xt")
            nc.sync.dma_start(xt[:], x[t * P : (t + 1) * P, :])
            xT_ps = ps.tile([P, P], F32, space="PSUM", tag="xT")
            nc.tensor.transpose(xT_ps[:], xt[:], ident[:])
            xT = sb.tile([P, P], F32, tag="xTs")
            nc.scalar.copy(xT[:], xT_ps[:])
            h_ps = ps.tile([P, F + 8], F32, space="PSUM", tag="hps")
            nc.tensor.matmul(h_ps[:], lhsT=xT[:], rhs=w1t[:], start=True, stop=True)
            _table_write(nc, sb, h_ps, tab, t)
    nc.compile()
    return pr


def build_launch_agg(sch, layer, b3=0.0, stage=99):
    """layer=1: L1 agg -> table2 (bf16 rows); layer=2: L2 agg -> table3
    (f32 rows); layer=3: L3 agg -> output.  stage: debug early-exit level."""
    pr = Prog()
    nc = pr.nc
    TOTCH = sch["TOTCH"]
    last = layer == 3
    roww = ROWW3 if last else ROWW
    tab_dt = F32 if last else BF16
    table = pr.inp("table", [N_NODES, roww], tab_dt)
    mytab = pr.inp("mytab", [NSP, roww], tab_dt)
    idx16 = pr.inp("idx16", [P, TOTCH * 8], I16)
    S_in = pr.inp("S", [P, TOTCH, P], FP8)
    ST_in = pr.inp("ST", [P, TOTCH, P], FP8)
    if layer == 1:
        naug = F + 8
        waug = pr.inp("waug", [F, naug], F32)
        tabout = pr.out("tabout", [NSP, ROWW], BF16)
    elif layer == 2:
        naug = 3
        waug = pr.inp("waug", [F, naug], F32)
        tabout = pr.out("tabout", [NSP, ROWW3], F32)
    else:
        outv = pr.out("outv", [NSP, 1], F32)
    if not last:
        wcol = pr.inp("wcol", [1, naug], F32)
        bias = pr.inp("bias", [P, 2], F32)
        nbias = pr.inp("nbias", [P, 2], F32)

    NAGG = (F + 4) if not last else 2
    nad = H if not last else 1
    with tile.TileContext(nc) as tc, ExitStack() as ctx:
        sb = ctx.enter_context(tc.tile_pool(name="sb", bufs=3))
        sbg = ctx.enter_context(tc.tile_pool(name="sbg", bufs=3))
        ps = ctx.enter_context(tc.tile_pool(name="ps", bufs=2, space="PSUM"))
        psa = ctx.enter_context(tc.tile_pool(name="psa", bufs=2, space="PSUM"))
        cpool = ctx.enter_context(tc.tile_pool(name="cp", bufs=1))
        ident = cpool.tile([P, P], F32)
        make_identity(nc, ident[:])
        if not last:
            waug_t = cpool.tile([P, F // P, naug], F32, tag="waug")
            for k in range(F // P):
                nc.sync.dma_start(waug_t[:, k, :], waug[k * P : (k + 1) * P, :])
            wcol_t = cpool.tile([1, naug], F32, tag="wcol")
            nc.sync.dma_start(wcol_t[:], wcol)
            bias_t = cpool.tile([P, 2], F32, tag="bias")
            nc.sync.dma_start(bias_t[:], bias)
            nbias_t = cpool.tile([P, 2], F32, tag="nbias")
            nc.sync.dma_start(nbias_t[:], nbias)
            negone = cpool.tile([1, P], F32, tag="negone")
            nc.vector.memset(negone[:], -1.0)

        for t in range(NT):
            NCH = int(sch["NCH"][t])
            LCH = int(sch["LCH"][t])
            HCH = int(sch["HCH"][t])
            c0 = int(sch["ch0"][t])
            idx_t = sb.tile([P, NCH * 8], I16, tag="idx")
            nc.sync.dma_start(idx_t[:], idx16[:, c0 * 8 : (c0 + NCH) * 8])
            s_t = sb.tile([P, NCH, P], FP8, tag="S")
            nc.sync.dma_start(s_t[:], S_in[:, c0 : c0 + NCH, :])
            st_t = sb.tile([P, NCH, P], FP8, tag="ST")
            nc.sync.dma_start(st_t[:], ST_in[:, c0 : c0 + NCH, :])
            g_t = sbg.tile([P, NCH, roww], tab_dt, tag="G")
            GMAX = 8  # 1024-descriptor SWDGE ring cap per dma_gather
            for a0, a1, base in ((0, LCH, 0), (LCH, NCH, HALF)):
                for j0 in range(a0, a1, GMAX):
                    j1 = min(j0 + GMAX, a1)
                    nc.gpsimd.dma_gather(
                        out_ap=g_t[:, j0:j1, :],
                        in_ap=table if base == 0 else table[base:, :],
                        idxs_ap=idx_t[:, j0 * 8 : j1 * 8],
                        num_idxs=(j1 - j0) * P, num_idxs_reg=(j1 - j0) * P,
                        elem_size=roww)
            # adtile: own-shard a_dst rows for this tile, cast to fp16
            if not last:
                adraw = sb.tile([P, 16], BF16, tag="adraw")
                nc.sync.dma_start(adraw[:], mytab[t * P : (t + 1) * P, F : F + 16])
                ad_f32 = adraw[:].bitcast(F32)[:, 4:8]
            else:
                adraw = sb.tile([P, 4], F32, tag="adraw")
                nc.sync.dma_start(adraw[:], mytab[t * P : (t + 1) * P, 0:4])
                ad_f32 = adraw[:, 2:3]
            adt = sb.tile([P, nad], FP16, tag="adt")
            nc.vector.tensor_copy(adt[:], ad_f32)
            if stage == 0:
                trow = sb.tile([P, ROWW if not last else ROWW3], tab_dt, tag="trow")
                nc.vector.tensor_copy(trow[:], g_t[:, 0, :])
                nc.sync.dma_start(tabout[t * P : (t + 1) * P, :], trow[:])
                continue
            # a_dst expansion matmuls (per chunk) into one PSUM strip
            zps = ps.tile([P, NCH * nad], F32, space="PSUM", tag="zps")
            for j in range(NCH):
                nc.tensor.matmul(zps[:, j * nad : (j + 1) * nad],
                                 lhsT=st_t[:, j, :], rhs=adt[:],
                                 start=True, stop=True)
            # z = a_src + expanded a_dst ; e = exp(prelu(z))
            if not last:
                as_ap = g_t[:, :, F : F + 16].bitcast(F32)[:, :, 0:4]
            else:
                as_ap = g_t[:, :, 1:2]
            z_t = sb.tile([P, NCH, nad], F32, tag="z")
            nc.vector.tensor_tensor(
                out=z_t[:], in0=as_ap,
                in1=zps[:].rearrange("p (c h) -> p c h", h=nad),
                op=mybir.AluOpType.add)
            l_t = sb.tile([P, NCH, nad], F32, tag="l")
            nc.scalar.activation(l_t[:], z_t[:],
                                 mybir.ActivationFunctionType.Prelu,
                                 alpha=NEG_SLOPE)
            e_t = sb.tile([P, NCH, nad], F32, tag="e")
            nc.scalar.activation(e_t[:], l_t[:],
                                 mybir.ActivationFunctionType.Exp)
            if stage == 1:
                trow = sb.tile([P, ROWW if not last else ROWW3], tab_dt, tag="trow")
                nc.vector.memset(trow[:], 0.0)
                nc.vector.tensor_copy(trow[:, 0 : NCH * nad], e_t[:])
                nc.sync.dma_start(tabout[t * P : (t + 1) * P, :], trow[:])
                continue
            # weighted messages rhs = [e*h | e]
            eg_t = sbg.tile([P, NCH, NAGG], BF16, tag="eg")
            if not last:
                nc.vector.tensor_tensor(
                    out=eg_t[:, :, 0:F].rearrange("p c (h f) -> p c h f", h=H),
                    in0=g_t[:, :, 0:F].rearrange("p c (h f) -> p c h f", h=H),
                    in1=e_t[:].broadcast_to([P, NCH, H, F // H]),
                    op=mybir.AluOpType.mult)
                nc.vector.tensor_copy(eg_t[:, :, F : F + 4], e_t[:])
            else:
                nc.vector.tensor_tensor(
                    out=eg_t[:, :, 0:1], in0=g_t[:, :, 0:1], in1=e_t[:],
                    op=mybir.AluOpType.mult)
                nc.vector.tensor_copy(eg_t[:, :, 1:2], e_t[:])
            if stage == 2:
                trow = sb.tile([P, ROWW if not last else ROWW3], tab_dt, tag="trow")
                nc.vector.tensor_copy(trow[:, 0:NAGG], eg_t[:, 0, :])
                nc.vector.memset(trow[:, NAGG:], 0.0)
                nc.sync.dma_start(tabout[t * P : (t + 1) * P, :], trow[:])
                continue
            # aggregation matmuls
            agg = psa.tile([P, NAGG], F32, space="PSUM", tag="agg")
            for j in range(NCH):
                nc.tensor.matmul(agg[:], lhsT=s_t[:, j, :], rhs=eg_t[:, j, :],
                                 start=(j == 0), stop=(j == NCH - 1))
            if stage == 3:
                trow = sb.tile([P, ROWW if not last else ROWW3], tab_dt, tag="trow")
                nc.vector.tensor_copy(trow[:, 0:NAGG], agg[:])
                nc.vector.memset(trow[:, NAGG:], 0.0)
                nc.sync.dma_start(tabout[t * P : (t + 1) * P, :], trow[:])
                continue
            # epilogue
            if last:
                den = sb.tile([P, 1], F32, tag="den")
                nc.vector.tensor_scalar_add(den[:], agg[:, 1:2], 1e-16)
                r_t = sb.tile([P, 1], F32, tag="r")
                nc.vector.reciprocal(r_t[:], den[:])
                o_t = sb.tile([P, 1], F32, tag="o")
                nc.vector.tensor_tensor(out=o_t[:], in0=agg[:, 0:1], in1=r_t[:],
                                        op=mybir.AluOpType.mult)
                if b3 != 0.0:
                    nc.vector.tensor_scalar_add(o_t[:], o_t[:], float(b3))
                nc.sync.dma_start(outv[t * P : (t + 1) * P, :], o_t[:])
                continue
            den = sb.tile([P, H], F32, tag="den")
            nc.vector.tensor_scalar_add(den[:], agg[:, F : F + 4], 1e-16)
            r_t = sb.tile([P, H], F32, tag="r")
            nc.vector.reciprocal(r_t[:], den[:])
            xn = sb.tile([P, F], F32, tag="xn")
            for h in range(H):
                nc.scalar.mul(xn[:, h * 64 : (h + 1) * 64],
                              agg[:, h * 64 : (h + 1) * 64], r_t[:, h : h + 1])
            # ELU(x + b) = relu(z+b) + exp(min(z+b,0)) - 1, -1 folded into
            # matmul via negone row; done on transposed tiles (bias per part.)
            h_ps = psa.tile([P, naug], F32, space="PSUM", tag="hps")
            for k in range(2):
                xT_ps = ps.tile([P, P], F32, space="PSUM", tag="xT")
                nc.tensor.transpose(xT_ps[:], xn[:, k * P : (k + 1) * P], ident[:])
                p_t = sb.tile([P, P], F32, tag="p")
                nc.scalar.activation(p_t[:], xT_ps[:],
                                     mybir.ActivationFunctionType.Relu,
                                     bias=bias_t[:, k : k + 1])
                m_t = sb.tile([P, P], F32, tag="m")
                nc.scalar.activation(m_t[:], xT_ps[:],
                                     mybir.ActivationFunctionType.Relu,
                                     bias=nbias_t[:, k : k + 1], scale=-1.0)
                q_t = sb.tile([P, P], F32, tag="q")
                nc.scalar.activation(q_t[:], m_t[:],
                                     mybir.ActivationFunctionType.Exp,
                                     scale=-1.0)
                xe_t = sb.tile([P, P], F32, tag="xe")
                nc.vector.tensor_tensor(out=xe_t[:], in0=p_t[:], in1=q_t[:],
                                        op=mybir.AluOpType.add)
                nc.tensor.matmul(h_ps[:], lhsT=xe_t[:], rhs=waug_t[:, k, :],
                                 start=(k == 0), stop=False)
            nc.tensor.matmul(h_ps[:], lhsT=negone[:], rhs=wcol_t[:],
                             start=False, stop=True)
            if layer == 1:
                _table_write(nc, sb, h_ps, tabout, t)
            else:
                trow = sb.tile([P, ROWW3], F32, tag="trow")
                nc.vector.memset(trow[:], 0.0)
                nc.vector.tensor_copy(trow[:, 0:3], h_ps[:, 0:3])
                nc.sync.dma_start(tabout[t * P : (t + 1) * P, :], trow[:])
    nc.compile()
    return pr


# --------------------------------------------------------------- the kernel

LAST_TIMES = {}


def _run(pr, in_maps, tag=None):
    if tag is not None:
        try:
            from concourse.timeline_sim import TimelineSim
            LAST_TIMES[tag] = TimelineSim(pr.nc, trace=False).simulate() / 1e9
        except Exception:
            pass
    res = bass_utils.run_bass_kernel_spmd(
        pr.nc, in_maps, core_ids=list(range(N_CORES)))
    return res.results


def _blockdiag_A(a_src, a_dst):
    Hh, C = a_src.shape
    A = np.zeros((Hh * C, 2 * Hh), np.float32)
    for h in range(Hh):
        A[h * C : (h + 1) * C, h] = a_src[h]
        A[h * C : (h + 1) * C, Hh + h] = a_dst[h]
    return A


def _pad_rows(a, n):
    out = np.zeros((n,) + a.shape[1:], a.dtype)
    out[: len(a)] = a
    return out


def kernel(x, edge_index, W1, a_src1, a_dst1, b1, W2, a_src2, a_dst2, b2,
           W3, a_src3, a_dst3, b3):
    x = np.asarray(x, np.float32)
    ei = np.asarray(edge_index)
    loops = np.arange(N_NODES, dtype=np.int64)
    src = np.concatenate([ei[0], loops]).astype(np.int64)
    dst = np.concatenate([ei[1], loops]).astype(np.int64)

    sch = build_schedule(src, dst)

    W1 = np.asarray(W1, np.float32); W2 = np.asarray(W2, np.float32)
    W3 = np.asarray(W3, np.float32)
    Waug1 = np.concatenate(
        [W1, W1 @ _blockdiag_A(np.asarray(a_src1), np.asarray(a_dst1))], 1)
    Waug2 = np.concatenate(
        [W2, W2 @ _blockdiag_A(np.asarray(a_src2), np.asarray(a_dst2))], 1)
    Waug3 = np.concatenate(
        [W3, W3 * float(np.asarray(a_src3)[0, 0]),
         W3 * float(np.asarray(a_dst3)[0, 0])], 1).astype(np.float32)
    wcol2 = Waug2.sum(0, keepdims=True).astype(np.float32)
    wcol3 = Waug3.sum(0, keepdims=True).astype(np.float32)
    b1T = np.asarray(b1, np.float32).reshape(2, P).T.copy()
    b2T = np.asarray(b2, np.float32).reshape(2, P).T.copy()

    # launch A: table1 from x
    prA = build_launch_A()
    inA = []
    for c in range(N_CORES):
        inA.append(dict(x=_pad_rows(x[c * NS : (c + 1) * NS], NSP), w1=Waug1))
    resA = _run(prA, inA, tag="A")
    tab1 = np.concatenate([resA[c]["tab"][:NS] for c in range(N_CORES)], 0)
    tab1 = np.ascontiguousarray(tab1)

    # launch B: L1 aggregation -> table2
    prB = build_launch_agg(sch, 1)
    inB = [dict(table=tab1, mytab=_pad_rows(tab1[c * NS : (c + 1) * NS], NSP),
                idx16=sch["idx16"][c], S=sch["S"][c], ST=sch["ST"][c],
                waug=Waug2, wcol=wcol2, bias=b1T, nbias=np.ascontiguousarray(-b1T))
           for c in range(N_CORES)]
    resB = _run(prB, inB, tag="B")
    tab2 = np.ascontiguousarray(
        np.concatenate([resB[c]["tabout"][:NS] for c in range(N_CORES)], 0))

    # launch C: L2 aggregation -> table3
    prC = build_launch_agg(sch, 2)
    inC = [dict(table=tab2, mytab=_pad_rows(tab2[c * NS : (c + 1) * NS], NSP),
                idx16=sch["idx16"][c], S=sch["S"][c], ST=sch["ST"][c],
                waug=Waug3, wcol=wcol3, bias=b2T, nbias=np.ascontiguousarray(-b2T))
           for c in range(N_CORES)]
    resC = _run(prC, inC, tag="C")
    tab3 = np.ascontiguousarray(
        np.concatenate([resC[c]["tabout"][:NS] for c in range(N_CORES)], 0))

    # launch D: L3 aggregation -> out
    prD = build_launch_agg(sch, 3, b3=float(np.asarray(b3).reshape(-1)[0]))
    inD = [dict(table=tab3, mytab=_pad_rows(tab3[c * NS : (c + 1) * NS], NSP),
                idx16=sch["idx16"][c], S=sch["S"][c], ST=sch["ST"][c])
           for c in range(N_CORES)]
    resD = _run(prD, inD, tag="D")
    out = np.concatenate([resD[c]["outv"][:NS] for c in range(N_CORES)], 0)
    return np.ascontiguousarray(out.astype(np.float32))


# revision 14
# speedup vs baseline: 1.0744x; 1.0336x over previous
"""GAT (3-layer, PyG-style) on 8 Trainium2 NeuronCores via Bass/Tile.

Strategy (dst-sharded graph parallel):
  - Nodes sharded 8 ways by destination; edges partitioned by dst shard,
    sorted by dst, grouped into 128-dst "tiles" and 128-edge "chunks".
  - Per layer, a node table [N, row] holds [h(bf16) | a_src(f32) | a_dst(f32)]
    in HBM on every core (stitched on host between launches = halo exchange).
  - Per-edge source rows fetched with gpsimd dma_gather (int16 idx, low/high
    table-half split to fit int16).
  - a_dst[dst] expanded per-edge with a host-built one-hot S_T (fp8) matmul.
  - logits -> Prelu -> Exp on ACT; e * h[src] on DVE; segment-sum via
    host-built one-hot S (fp8) matmuls into PSUM, with e appended as extra
    rhs columns so segment softmax denominators come out of the same matmul.
  - Per-node epilogue: normalize, +bias, ELU, transpose, next-layer matmul
    (W pre-augmented with attention vectors so al_src/al_dst ride along).
"""
import numpy as np
import ml_dtypes
from contextlib import ExitStack

import concourse.bass as bass
import concourse.tile as tile
from concourse import bacc, mybir
from concourse import bass_utils
from concourse.masks import make_identity

P = 128
N_NODES = 50000
N_EDGES = 650000
NEG_SLOPE = 0.2
N_CORES = 8
NS = N_NODES // N_CORES            # 6250 nodes per shard
NT = (NS + P - 1) // P             # 49 dst tiles per core
NSP = NT * P                       # padded shard nodes (6272)
HALF = 32768                       # int16 gather limit -> low/high split
ROWW = 384                         # bf16 slots per table row (768B)
ROWW3 = 64                         # f32 per layer-3 table row (256B)
F = 256                            # feature width (H*C)
H = 4

FP8 = mybir.dt.float8e4
BF16 = mybir.dt.bfloat16
FP16 = mybir.dt.float16
F32 = mybir.dt.float32
I16 = mybir.dt.int16


# ----------------------------------------------------------------- host prep

def _wrap16(idx_flat):
    """[n] int array -> [128, n//16] int16 (16-partition wrap, replicated)."""
    n = len(idx_flat)
    a = np.asarray(idx_flat, dtype=np.int16).reshape(n // 16, 16).T
    return np.tile(a, (8, 1))


def build_schedule(src, dst):
    """Partition edges by dst shard / dst tile / src half, pad to chunks."""
    order = np.argsort(dst, kind="stable")
    src = src[order]
    dst = dst[order]
    core_of = dst // NS
    core_starts = np.searchsorted(core_of, np.arange(N_CORES + 1))

    per_core = []
    for c in range(N_CORES):
        s0, s1 = core_starts[c], core_starts[c + 1]
        cs, cd = src[s0:s1], dst[s0:s1] - c * NS
        tile_of = cd // P
        tile_starts = np.searchsorted(tile_of, np.arange(NT + 1))
        tiles = []
        for t in range(NT):
            t0, t1 = tile_starts[t], tile_starts[t + 1]
            ts_, td_ = cs[t0:t1], cd[t0:t1] - t * P
            lo = ts_ < HALF
            tiles.append(((ts_[lo], td_[lo]), (ts_[~lo] - HALF, td_[~lo])))
        per_core.append(tiles)

    LCH = np.zeros(NT, np.int64)
    HCH = np.zeros(NT, np.int64)
    for t in range(NT):
        for c in range(N_CORES):
            (ls, _), (hs, _) = per_core[c][t]
            LCH[t] = max(LCH[t], -(-len(ls) // P))
            HCH[t] = max(HCH[t], -(-len(hs) // P))
        LCH[t] = max(LCH[t], 1)
    NCH = LCH + HCH
    ch0 = np.concatenate([[0], np.cumsum(NCH)])
    TOTCH = int(ch0[-1])

    idx16 = np.zeros((N_CORES, P, TOTCH * 8), np.int16)
    S = np.zeros((N_CORES, P, TOTCH, P), ml_dtypes.float8_e4m3)
    ST = np.zeros((N_CORES, P, TOTCH, P), ml_dtypes.float8_e4m3)
    one = ml_dtypes.float8_e4m3(1.0)
    for c in range(N_CORES):
        for t in range(NT):
            for half, (es, ed) in enumerate(per_core[c][t]):
                nch = int((LCH[t], HCH[t])[half])
                if nch == 0:
                    continue
                base = int(ch0[t]) + (int(LCH[t]) if half else 0)
                n = nch * P
                e_pad = np.zeros(n, np.int64)
                e_pad[: len(es)] = es
                if len(es):
                    k = np.arange(len(es))
                    S[c, k % P, base + k // P, ed] = one
                    ST[c, ed, base + k // P, k % P] = one
                idx16[c, :, base * 8 : (base + nch) * 8] = _wrap16(e_pad)
    return dict(LCH=LCH, HCH=HCH, NCH=NCH, ch0=ch0, TOTCH=TOTCH,
                idx16=idx16, S=S, ST=ST)


# ------------------------------------------------------------- bass builders

class Prog:
    def __init__(self):
        self.nc = bacc.Bacc("TRN2", target_bir_lowering=False, debug=False,
                            num_devices=N_CORES)
        self.in_aps = {}
        self.out_aps = {}

    def inp(self, name, shape, dt):
        ap = self.nc.dram_tensor(name, list(shape), dt, kind="ExternalInput").ap()
        self.in_aps[name] = ap
        return ap

    def out(self, name, shape, dt):
        ap = self.nc.dram_tensor(name, list(shape), dt, kind="ExternalOutput").ap()
        self.out_aps[name] = ap
        return ap


def _table_write(nc, sb, h_ps, tabout, t):
    """PSUM [128, 264] -> table row tile (bf16 h + raw f32 as/ad) -> HBM."""
    trow = sb.tile([P, ROWW], BF16, tag="trow")
    nc.vector.memset(trow[:, F + 16 : ROWW], 0.0)
    nc.scalar.copy(trow[:, 0:F], h_ps[:, 0:F])
    nc.vector.tensor_copy(trow[:, F : F + 16].bitcast(F32), h_ps[:, F : F + 8])
    nc.sync.dma_start(tabout[t * P : (t + 1) * P, :], trow[:])


def build_launch_A():
    """x_shard @ Waug1 -> table1 rows [h1 | as1 | ad1]."""
    pr = Prog()
    nc = pr.nc
    x = pr.inp("x", [NSP, P], F32)
    w1 = pr.inp("w1", [P, F + 8], F32)
    tab = pr.out("tab", [NSP, ROWW], BF16)
    with tile.TileContext(nc) as tc, ExitStack() as ctx:
        sb = ctx.enter_context(tc.tile_pool(name="sb", bufs=5))
        ps = ctx.enter_context(tc.tile_pool(name="ps", bufs=4, space="PSUM"))
        cpool = ctx.enter_context(tc.tile_pool(name="cp", bufs=1))
        ident = cpool.tile([P, P], F32)
        make_identity(nc, ident[:])
        w1t = cpool.tile([P, F + 8], F32)
        nc.sync.dma_start(w1t[:], w1)
        B4 = 7  # tiles per batched x-load / table-store DMA
        for t0 in range(0, NT, B4):
            nb = min(B4, NT - t0)
            xt = sb.tile([P, B4, P], F32, tag="xt")
            nc.sync.dma_start(
                xt[:, 0:nb, :],
                x[t0 * P : (t0 + nb) * P, :].rearrange("(b p) f -> p b f", p=P))
            trow = sb.tile([P, B4, ROWW], BF16, tag="trow")
            nc.vector.memset(trow[:], 0.0)
            for j in range(nb):
                t = t0 + j
                xT_ps = ps.tile([P, P], F32, space="PSUM", tag="xT")
                nc.tensor.transpose(xT_ps[:], xt[:, j, :], ident[:])
                xT = sb.tile([P, P], F32, tag="xTs")
                nc.scalar.copy(xT[:], xT_ps[:])
                h_ps = ps.tile([P, F + 8], F32, space="PSUM", tag="hps")
                nc.tensor.matmul(h_ps[:], lhsT=xT[:], rhs=w1t[:],
                                 start=True, stop=True)
                nc.scalar.copy(trow[:, j, 0:F], h_ps[:, 0:F])
                nc.vector.tensor_copy(
                    trow[:, j, F : F + 16].bitcast(F32), h_ps[:, F : F + 8])
            nc.sync.dma_start(
                tab[t0 * P : (t0 + nb) * P, :].rearrange("(b p) f -> p b f", p=P),
                trow[:, 0:nb, :])
    nc.compile()
    return pr


def build_launch_agg(sch, layer, b3=0.0, stage=99):
    """layer=1: L1 agg -> table2 (bf16 rows); layer=2: L2 agg -> table3
    (f32 rows); layer=3: L3 agg -> output.  stage: debug early-exit level."""
    pr = Prog()
    nc = pr.nc
    TOTCH = sch["TOTCH"]
    last = layer == 3
    roww = ROWW3 if last else ROWW
    tab_dt = F32 if last else BF16
    table = pr.inp("table", [N_NODES, roww], tab_dt)
    mytab = pr.inp("mytab", [NSP, roww], tab_dt)
    idx16 = pr.inp("idx16", [P, TOTCH * 8], I16)
    S_in = pr.inp("S", [P, TOTCH, P], FP8)
    ST_in = pr.inp("ST", [P, TOTCH, P], FP8)
    if layer == 1:
        naug = F + 8
        waug = pr.inp("waug", [F, naug], F32)
        tabout = pr.out("tabout", [NSP, ROWW], BF16)
    elif layer == 2:
        naug = 3
        waug = pr.inp("waug", [F, naug], F32)
        tabout = pr.out("tabout", [NSP, ROWW3], F32)
    else:
        outv = pr.out("outv", [NSP, 1], F32)
    if not last:
        wcol = pr.inp("wcol", [1, naug], F32)
        bias = pr.inp("bias", [P, 2], F32)
        nbias = pr.inp("nbias", [P, 2], F32)

    NAGG = (F + 4) if not last else 2
    nad = H if not last else 1
    with tile.TileContext(nc) as tc, ExitStack() as ctx:
        sb = ctx.enter_context(tc.tile_pool(name="sb", bufs=3))
        sbg = ctx.enter_context(tc.tile_pool(name="sbg", bufs=3))
        ps = ctx.enter_context(tc.tile_pool(name="ps", bufs=2, space="PSUM"))
        psa = ctx.enter_context(tc.tile_pool(name="psa", bufs=2, space="PSUM"))
        cpool = ctx.enter_context(tc.tile_pool(name="cp", bufs=1))
        ident = cpool.tile([P, P], F32)
        make_identity(nc, ident[:])
        if not last:
            waug_t = cpool.tile([P, F // P, naug], F32, tag="waug")
            for k in range(F // P):
                nc.sync.dma_start(waug_t[:, k, :], waug[k * P : (k + 1) * P, :])
            wcol_t = cpool.tile([1, naug], F32, tag="wcol")
            nc.sync.dma_start(wcol_t[:], wcol)
            bias_t = cpool.tile([P, 2], F32, tag="bias")
            nc.sync.dma_start(bias_t[:], bias)
            nbias_t = cpool.tile([P, 2], F32, tag="nbias")
            nc.sync.dma_start(nbias_t[:], nbias)
            negone = cpool.tile([1, P], F32, tag="negone")
            nc.vector.memset(negone[:], -1.0)
        else:
            obuf = cpool.tile([P, NT], F32, tag="obuf")

        for t in range(NT):
            NCH = int(sch["NCH"][t])
            LCH = int(sch["LCH"][t])
            HCH = int(sch["HCH"][t])
            c0 = int(sch["ch0"][t])
            idx_t = sb.tile([P, NCH * 8], I16, tag="idx")
            nc.sync.dma_start(idx_t[:], idx16[:, c0 * 8 : (c0 + NCH) * 8])
            s_t = sb.tile([P, NCH, P], FP8, tag="S")
            nc.sync.dma_start(s_t[:], S_in[:, c0 : c0 + NCH, :])
            st_t = sb.tile([P, NCH, P], FP8, tag="ST")
            nc.sync.dma_start(st_t[:], ST_in[:, c0 : c0 + NCH, :])
            g_t = sbg.tile([P, NCH, roww], tab_dt, tag="G")
            GMAX = 8  # 1024-descriptor SWDGE ring cap per dma_gather
            for a0, a1, base in ((0, LCH, 0), (LCH, NCH, HALF)):
                for j0 in range(a0, a1, GMAX):
                    j1 = min(j0 + GMAX, a1)
                    nc.gpsimd.dma_gather(
                        out_ap=g_t[:, j0:j1, :],
                        in_ap=table if base == 0 else table[base:, :],
                        idxs_ap=idx_t[:, j0 * 8 : j1 * 8],
                        num_idxs=(j1 - j0) * P, num_idxs_reg=(j1 - j0) * P,
                        elem_size=roww)
            # adtile: own-shard a_dst rows for this tile, cast to fp16
            if not last:
                adraw = sb.tile([P, 16], BF16, tag="adraw")
                nc.sync.dma_start(adraw[:], mytab[t * P : (t + 1) * P, F : F + 16])
                ad_f32 = adraw[:].bitcast(F32)[:, 4:8]
            else:
                adraw = sb.tile([P, 4], F32, tag="adraw")
                nc.sync.dma_start(adraw[:], mytab[t * P : (t + 1) * P, 0:4])
                ad_f32 = adraw[:, 2:3]
            adt = sb.tile([P, nad], FP16, tag="adt")
            nc.vector.tensor_copy(adt[:], ad_f32)
            if stage == 0:
                trow = sb.tile([P, ROWW if not last else ROWW3], tab_dt, tag="trow")
                nc.vector.tensor_copy(trow[:], g_t[:, 0, :])
                nc.sync.dma_start(tabout[t * P : (t + 1) * P, :], trow[:])
                continue
            # a_dst expansion matmuls (per chunk) into one PSUM strip
            zps = ps.tile([P, NCH * nad], F32, space="PSUM", tag="zps")
            for j in range(NCH):
                nc.tensor.matmul(zps[:, j * nad : (j + 1) * nad],
                                 lhsT=st_t[:, j, :], rhs=adt[:],
                                 start=True, stop=True)
            # z = a_src + expanded a_dst ; e = exp(prelu(z))
            if not last:
                as_ap = g_t[:, :, F : F + 16].bitcast(F32)[:, :, 0:4]
            else:
                as_ap = g_t[:, :, 1:2]
            z_t = sb.tile([P, NCH, nad], F32, tag="z")
            nc.vector.tensor_tensor(
                out=z_t[:], in0=as_ap,
                in1=zps[:].rearrange("p (c h) -> p c h", h=nad),
                op=mybir.AluOpType.add)
            l_t = sb.tile([P, NCH, nad], F32, tag="l")
            nc.scalar.activation(l_t[:], z_t[:],
                                 mybir.ActivationFunctionType.Prelu,
                                 alpha=NEG_SLOPE)
            e_t = sb.tile([P, NCH, nad], F32, tag="e")
            nc.scalar.activation(e_t[:], l_t[:],
                                 mybir.ActivationFunctionType.Exp)
            if stage == 1:
                trow = sb.tile([P, ROWW if not last else ROWW3], tab_dt, tag="trow")
                nc.vector.memset(trow[:], 0.0)
                nc.vector.tensor_copy(trow[:, 0 : NCH * nad], e_t[:])
                nc.sync.dma_start(tabout[t * P : (t + 1) * P, :], trow[:])
                continue
            # weighted messages rhs = [e*h | e]
            eg_t = sbg.tile([P, NCH, NAGG], BF16, tag="eg")
            if not last:
                nc.vector.tensor_tensor(
                    out=eg_t[:, :, 0:F].rearrange("p c (h f) -> p c h f", h=H),
                    in0=g_t[:, :, 0:F].rearrange("p c (h f) -> p c h f", h=H),
                    in1=e_t[:].broadcast_to([P, NCH, H, F // H]),
                    op=mybir.AluOpType.mult)
                nc.vector.tensor_copy(eg_t[:, :, F : F + 4], e_t[:])
            else:
                nc.vector.tensor_tensor(
                    out=eg_t[:, :, 0:1], in0=g_t[:, :, 0:1], in1=e_t[:],
                    op=mybir.AluOpType.mult)
                nc.vector.tensor_copy(eg_t[:, :, 1:2], e_t[:])
            if stage == 2:
                trow = sb.tile([P, ROWW if not last else ROWW3], tab_dt, tag="trow")
                nc.vector.tensor_copy(trow[:, 0:NAGG], eg_t[:, 0, :])
                nc.vector.memset(trow[:, NAGG:], 0.0)
                nc.sync.dma_start(tabout[t * P : (t + 1) * P, :], trow[:])
                continue
            # aggregation matmuls
            agg = psa.tile([P, NAGG], F32, space="PSUM", tag="agg")
            for j in range(NCH):
                nc.tensor.matmul(agg[:], lhsT=s_t[:, j, :], rhs=eg_t[:, j, :],
                                 start=(j == 0), stop=(j == NCH - 1))
            if stage == 3:
                trow = sb.tile([P, ROWW if not last else ROWW3], tab_dt, tag="trow")
                nc.vector.tensor_copy(trow[:, 0:NAGG], agg[:])
                nc.vector.memset(trow[:, NAGG:], 0.0)
                nc.sync.dma_start(tabout[t * P : (t + 1) * P, :], trow[:])
                continue
            # epilogue
            if last:
                den = sb.tile([P, 1], F32, tag="den")
                nc.vector.tensor_scalar_add(den[:], agg[:, 1:2], 1e-16)
                r_t = sb.tile([P, 1], F32, tag="r")
                nc.vector.reciprocal(r_t[:], den[:])
                nc.vector.tensor_tensor(out=obuf[:, t : t + 1], in0=agg[:, 0:1],
                                        in1=r_t[:], op=mybir.AluOpType.mult)
                if b3 != 0.0:
                    nc.vector.tensor_scalar_add(
                        obuf[:, t : t + 1], obuf[:, t : t + 1], float(b3))
                continue
            den = sb.tile([P, H], F32, tag="den")
            nc.vector.tensor_scalar_add(den[:], agg[:, F : F + 4], 1e-16)
            r_t = sb.tile([P, H], F32, tag="r")
            nc.vector.reciprocal(r_t[:], den[:])
            xn = sb.tile([P, F], F32, tag="xn")
            for h in range(H):
                nc.scalar.mul(xn[:, h * 64 : (h + 1) * 64],
                              agg[:, h * 64 : (h + 1) * 64], r_t[:, h : h + 1])
            # ELU(x + b) = relu(z+b) + exp(min(z+b,0)) - 1, -1 folded into
            # matmul via negone row; done on transposed tiles (bias per part.)
            h_ps = psa.tile([P, naug], F32, space="PSUM", tag="hps")
            for k in range(2):
                xT_ps = ps.tile([P, P], F32, space="PSUM", tag="xT")
                nc.tensor.transpose(xT_ps[:], xn[:, k * P : (k + 1) * P], ident[:])
                p_t = sb.tile([P, P], F32, tag="p")
                nc.scalar.activation(p_t[:], xT_ps[:],
                                     mybir.ActivationFunctionType.Relu,
                                     bias=bias_t[:, k : k + 1])
                m_t = sb.tile([P, P], F32, tag="m")
                nc.scalar.activation(m_t[:], xT_ps[:],
                                     mybir.ActivationFunctionType.Relu,
                                     bias=nbias_t[:, k : k + 1], scale=-1.0)
                q_t = sb.tile([P, P], F32, tag="q")
                nc.scalar.activation(q_t[:], m_t[:],
                                     mybir.ActivationFunctionType.Exp,
                                     scale=-1.0)
                xe_t = sb.tile([P, P], F32, tag="xe")
                nc.vector.tensor_tensor(out=xe_t[:], in0=p_t[:], in1=q_t[:],
                                        op=mybir.AluOpType.add)
                nc.tensor.matmul(h_ps[:], lhsT=xe_t[:], rhs=waug_t[:, k, :],
                                 start=(k == 0), stop=False)
            nc.tensor.matmul(h_ps[:], lhsT=negone[:], rhs=wcol_t[:],
                             start=False, stop=True)
            if layer == 1:
                _table_write(nc, sb, h_ps, tabout, t)
            else:
                trow = sb.tile([P, ROWW3], F32, tag="trow")
                nc.vector.memset(trow[:], 0.0)
                nc.vector.tensor_copy(trow[:, 0:3], h_ps[:, 0:3])
                nc.sync.dma_start(tabout[t * P : (t + 1) * P, :], trow[:])
        if last:
            nc.sync.dma_start(
                outv.rearrange("(t p) o -> p t o", p=P).squeeze(-1), obuf[:])
    nc.compile()
    return pr


# --------------------------------------------------------------- the kernel

LAST_TIMES = {}


def _run(pr, in_maps, tag=None):
    if tag is not None:
        try:
            from concourse.timeline_sim import TimelineSim
            LAST_TIMES[tag] = TimelineSim(pr.nc, trace=False).simulate() / 1e9
        except Exception:
            pass
    res = bass_utils.run_bass_kernel_spmd(
        pr.nc, in_maps, core_ids=list(range(N_CORES)))
    return res.results


def _blockdiag_A(a_src, a_dst):
    Hh, C = a_src.shape
    A = np.zeros((Hh * C, 2 * Hh), np.float32)
    for h in range(Hh):
        A[h * C : (h + 1) * C, h] = a_src[h]
        A[h * C : (h + 1) * C, Hh + h] = a_dst[h]
    return A


def _pad_rows(a, n):
    out = np.zeros((n,) + a.shape[1:], a.dtype)
    out[: len(a)] = a
    return out


def kernel(x, edge_index, W1, a_src1, a_dst1, b1, W2, a_src2, a_dst2, b2,
           W3, a_src3, a_dst3, b3):
    x = np.asarray(x, np.float32)
    ei = np.asarray(edge_index)
    loops = np.arange(N_NODES, dtype=np.int64)
    src = np.concatenate([ei[0], loops]).astype(np.int64)
    dst = np.concatenate([ei[1], loops]).astype(np.int64)

    sch = build_schedule(src, dst)

    W1 = np.asarray(W1, np.float32); W2 = np.asarray(W2, np.float32)
    W3 = np.asarray(W3, np.float32)
    Waug1 = np.concatenate(
        [W1, W1 @ _blockdiag_A(np.asarray(a_src1), np.asarray(a_dst1))], 1)
    Waug2 = np.concatenate(
        [W2, W2 @ _blockdiag_A(np.asarray(a_src2), np.asarray(a_dst2))], 1)
    Waug3 = np.concatenate(
        [W3, W3 * float(np.asarray(a_src3)[0, 0]),
         W3 * float(np.asarray(a_dst3)[0, 0])], 1).astype(np.float32)
    wcol2 = Waug2.sum(0, keepdims=True).astype(np.float32)
    wcol3 = Waug3.sum(0, keepdims=True).astype(np.float32)
    b1T = np.asarray(b1, np.float32).reshape(2, P).T.copy()
    b2T = np.asarray(b2, np.float32).reshape(2, P).T.copy()

    # launch A: table1 from x
    prA = build_launch_A()
    inA = []
    for c in range(N_CORES):
        inA.append(dict(x=_pad_rows(x[c * NS : (c + 1) * NS], NSP), w1=Waug1))
    resA = _run(prA, inA, tag="A")
    tab1 = np.concatenate([resA[c]["tab"][:NS] for c in range(N_CORES)], 0)
    tab1 = np.ascontiguousarray(tab1)

    # launch B: L1 aggregation -> table2
    prB = build_launch_agg(sch, 1)
    inB = [dict(table=tab1, mytab=_pad_rows(tab1[c * NS : (c + 1) * NS], NSP),
                idx16=sch["idx16"][c], S=sch["S"][c], ST=sch["ST"][c],
                waug=Waug2, wcol=wcol2, bias=b1T, nbias=np.ascontiguousarray(-b1T))
           for c in range(N_CORES)]
    resB = _run(prB, inB, tag="B")
    tab2 = np.ascontiguousarray(
        np.concatenate([resB[c]["tabout"][:NS] for c in range(N_CORES)], 0))

    # launch C: L2 aggregation -> table3
    prC = build_launch_agg(sch, 2)
    inC = [dict(table=tab2, mytab=_pad_rows(tab2[c * NS : (c + 1) * NS], NSP),
                idx16=sch["idx16"][c], S=sch["S"][c], ST=sch["ST"][c],
                waug=Waug3, wcol=wcol3, bias=b2T, nbias=np.ascontiguousarray(-b2T))
           for c in range(N_CORES)]
    resC = _run(prC, inC, tag="C")
    tab3 = np.ascontiguousarray(
        np.concatenate([resC[c]["tabout"][:NS] for c in range(N_CORES)], 0))

    # launch D: L3 aggregation -> out
    prD = build_launch_agg(sch, 3, b3=float(np.asarray(b3).reshape(-1)[0]))
    inD = [dict(table=tab3, mytab=_pad_rows(tab3[c * NS : (c + 1) * NS], NSP),
                idx16=sch["idx16"][c], S=sch["S"][c], ST=sch["ST"][c])
           for c in range(N_CORES)]
    resD = _run(prD, inD, tag="D")
    out = np.concatenate([resD[c]["outv"][:NS] for c in range(N_CORES)], 0)
    return np.ascontiguousarray(out.astype(np.float32))
